# revision 1
# baseline (speedup 1.0000x reference)
"""Trainium2 Bass kernel for nn_MlpMixer_18966575579742.

Complex-valued per-frequency (j) MLP:
  o1r = gelu(xr@w1[0] - xi@w1[1] + b1[0]);  o1i = gelu(xi@w1[0] + xr@w1[1] + b1[1])
  o2r = o1r@w2[0] - o1i@w2[1] + b2[0];      o2i = o1i@w2[0] + o1i@w2[1] + b2[1]
  (note: o2i intentionally uses o1i with BOTH w2[0] and w2[1], as in the source)

Sharding over 8 cores: 2 j-halves (13 each) x 4 batch-quarters (B=32 -> 512 rows).
Per-core dataflow (all fp32; fp32 matmul = 2 HW passes, so matmul count is the
whole game):
  - host pre-transposes x shards to [j, k, rows] and pre-sums xs = xr + xi,
    so every matmul operand is DMA'd straight into its streaming layout
  - L1 uses Gauss's 3-multiplication complex product (3 matmuls instead of 4):
    t1=(xr+xi)@w1[0], t2=xr@(w1[1]-w1[0]), t3=xi@(w1[0]+w1[1]) with the
    weight combinations built once per j on DVE; o1r_pre=t1-t3, o1i_pre=t1+t2
    combined on DVE, then exact-erf GELU + per-partition b1 bias on ScalarE
    (partitions = h, output kept transposed [h_chunk, rows])
  - L2 (w2 stationary, o1T moving, N=512): o2T [k'=128, rows] PSUM, accumulated
    via w2[0], -w2[1] (real) and w2[0]+w2[1] (imag) -- 3 matmuls per h_chunk
  - DVE drains PSUM with fused per-partition b2 bias (partitions = k')
  - output stays transposed [j, c, k', rows]; host does the final
    transpose + complex interleave (cheap numpy ops on gathered results)
  - biases are DMA'd in clean row-major staging tiles and PE-transposed once
  - DMA issue is spread across queues: x/out on sync, weights on scalar,
    bias staging on gpsimd (avoids head-of-line blocking at j boundaries)
"""

import sys

if "/opt/trn_rl_repo" not in sys.path:
    sys.path.insert(0, "/opt/trn_rl_repo")

import numpy as np

B, I, J, K, F = 128, 16, 26, 128, 4
H = K * F  # 512
NJG = 2  # j groups
NRG = 4  # row (batch) groups
JL = J // NJG  # 13 j per core
BL = B // NRG  # 32 batches per core
ROWS = BL * I  # 512 rows per core
NHC = H // 128  # 4 h-chunks

_cache = {}


def _build_nc():
    from contextlib import ExitStack

    import concourse.mybir as mybir
    import concourse.tile as tile
    from concourse import bacc
    from concourse.masks import make_identity

    f32 = mybir.dt.float32
    nc = bacc.Bacc(None)

    # x arrives pre-transposed from the host: [j, k, rows]; xs = xr + xi
    xr = nc.declare_dram_parameter("xr", [JL, K, ROWS], f32, isOutput=False)
    xi = nc.declare_dram_parameter("xi", [JL, K, ROWS], f32, isOutput=False)
    xs = nc.declare_dram_parameter("xs", [JL, K, ROWS], f32, isOutput=False)
    w1 = nc.declare_dram_parameter("w1", [2, JL, K, H], f32, isOutput=False)
    b1 = nc.declare_dram_parameter("b1", [2, JL, H], f32, isOutput=False)
    w2 = nc.declare_dram_parameter("w2", [2, JL, H, K], f32, isOutput=False)
    b2 = nc.declare_dram_parameter("b2", [2, JL, K], f32, isOutput=False)
    # transposed output: [j, c, k', rows]; host fixes layout
    out = nc.declare_dram_parameter("out", [JL, 2, K, ROWS], f32, isOutput=True)

    GELU = mybir.ActivationFunctionType.Gelu

    with tile.TileContext(nc) as tc, ExitStack() as ctx:
        const = ctx.enter_context(tc.tile_pool(name="const", bufs=1))
        w1p = ctx.enter_context(tc.tile_pool(name="w1p", bufs=3))
        w1np = ctx.enter_context(tc.tile_pool(name="w1np", bufs=2))
        w2p = ctx.enter_context(tc.tile_pool(name="w2p", bufs=3))
        w2xp = ctx.enter_context(tc.tile_pool(name="w2xp", bufs=2))
        xtp = ctx.enter_context(tc.tile_pool(name="xtp", bufs=3))
        o1p = ctx.enter_context(tc.tile_pool(name="o1p", bufs=2))
        cmb = ctx.enter_context(tc.tile_pool(name="cmb", bufs=2))
        outp = ctx.enter_context(tc.tile_pool(name="outp", bufs=4))
        ps1 = ctx.enter_context(tc.tile_pool(name="ps1", bufs=6, space="PSUM"))
        ps2 = ctx.enter_context(tc.tile_pool(name="ps2", bufs=2, space="PSUM"))

        identity = const.tile([128, 128], f32)
        make_identity(nc, identity)

        # biases: clean row-major staging DMA, then PE-transpose on chip.
        # b1s[(c j hc), p] rows are contiguous 512B; b1t[p, c, j, hc]
        b1s = const.tile([2 * JL * NHC, 128], f32)
        nc.gpsimd.dma_start(
            out=b1s, in_=b1.rearrange("c j (hc p) -> (c j hc) p", p=128)
        )
        b2s = const.tile([2 * JL, K], f32)
        nc.gpsimd.dma_start(out=b2s, in_=b2.rearrange("c j k -> (c j) k"))
        b1t = const.tile([128, 2, JL, NHC], f32)
        b2t = const.tile([128, 2, JL], f32)

        def bias1_stage():
            n1 = 2 * JL * NHC
            b1ps = ps2.tile([128, n1], f32, tag="ps2")
            nc.tensor.transpose(b1ps, b1s, identity[:n1, :n1])
            nc.vector.tensor_copy(b1t.rearrange("p c j hc -> p (c j hc)"), b1ps)

        def bias2_stage():
            n2 = 2 * JL
            b2ps = ps2.tile([128, n2], f32, tag="ps2")
            nc.tensor.transpose(b2ps, b2s, identity[:n2, :n2])
            nc.vector.tensor_copy(b2t.rearrange("p c j -> p (c j)"), b2ps)

        def load_weights(j):
            w1t = w1p.tile([128, 2, H], f32, tag="w1t")  # [k, c, h]
            # split per c so the first matmul's weights (c=0) land sooner
            nc.scalar.dma_start(out=w1t[:, 0], in_=w1[0, j])
            nc.scalar.dma_start(out=w1t[:, 1], in_=w1[1, j])
            # Gauss 3-mult complex product weights:
            # w1g[:,0] = w1[1]-w1[0];  w1g[:,1] = w1[0]+w1[1]
            w1g = w1np.tile([128, 2, H], f32, tag="w1n")
            nc.vector.tensor_sub(w1g[:, 0], w1t[:, 1], w1t[:, 0])
            nc.vector.tensor_add(w1g[:, 1], w1t[:, 0], w1t[:, 1])
            w2t = w2p.tile([128, 2, NHC, K], f32, tag="w2t")  # [p, c, hc, k']
            for c in range(2):
                nc.scalar.dma_start(
                    out=w2t[:, c],
                    in_=w2[c, j].rearrange("(hc p) k -> p hc k", p=128),
                )
            # w2x[:,0,hc] = -w2[1];  w2x[:,1,hc] = w2[0]+w2[1]
            w2x = w2xp.tile([128, 2, NHC, K], f32, tag="w2x")
            nc.vector.tensor_scalar_mul(w2x[:, 0], w2t[:, 1], -1.0)
            nc.vector.tensor_add(w2x[:, 1], w2t[:, 0], w2t[:, 1])
            return w1t, w1g, w2t, w2x

        for j in range(JL):
            w1t, w1g, w2t, w2x = load_weights(j)
            # xsum first: it feeds t1, the first matmul of the j iteration
            xsum = xtp.tile([128, ROWS], f32, tag="xsum")
            nc.sync.dma_start(out=xsum, in_=xs[j])
            xtr = xtp.tile([128, ROWS], f32, tag="xtr")
            nc.sync.dma_start(out=xtr, in_=xr[j])
            xti = xtp.tile([128, ROWS], f32, tag="xti")
            nc.sync.dma_start(out=xti, in_=xi[j])

            # --- layer 1 via Gauss: t1=(xr+xi)@w1[0], t2=xr@(w1[1]-w1[0]),
            # t3=xi@(w1[0]+w1[1]);  o1r=gelu(t1-t3+b1r), o1i=gelu(t1+t2+b1i)
            o1r = o1p.tile([128, NHC, ROWS], f32, tag="o1r")
            o1i = o1p.tile([128, NHC, ROWS], f32, tag="o1i")
            for hc in range(NHC):
                hs = slice(hc * 128, (hc + 1) * 128)
                t1 = ps1.tile([128, ROWS], f32, tag="ps1")
                t2 = ps1.tile([128, ROWS], f32, tag="ps1")
                t3 = ps1.tile([128, ROWS], f32, tag="ps1")
                nc.tensor.matmul(t1, w1t[:, 0, hs], xsum, start=True, stop=True)
                nc.tensor.matmul(t2, w1g[:, 0, hs], xtr, start=True, stop=True)
                nc.tensor.matmul(t3, w1g[:, 1, hs], xti, start=True, stop=True)
                if j == 0 and hc == 0:
                    # fills the PE pipe while the first GELU waits on b1t
                    bias1_stage()
                s1 = cmb.tile([128, ROWS], f32, tag="s1")
                nc.vector.tensor_copy(s1, t1)
                rp = cmb.tile([128, ROWS], f32, tag="rp")
                nc.vector.tensor_sub(rp, s1, t3)
                ip = cmb.tile([128, ROWS], f32, tag="ip")
                nc.vector.tensor_add(ip, s1, t2)
                nc.scalar.activation(
                    o1r[:, hc], rp, GELU, bias=b1t[:, 0, j, hc : hc + 1]
                )
                nc.scalar.activation(
                    o1i[:, hc], ip, GELU, bias=b1t[:, 1, j, hc : hc + 1]
                )

            if j == 0:
                bias2_stage()

            # --- layer 2 (w2 stationary; output transposed [k', rows]) ---
            p2r = ps2.tile([128, ROWS], f32, tag="ps2")
            p2i = ps2.tile([128, ROWS], f32, tag="ps2")
            for hc in range(NHC):
                last = hc == NHC - 1
                nc.tensor.matmul(
                    p2r, w2t[:, 0, hc], o1r[:, hc], start=(hc == 0), stop=False
                )
                nc.tensor.matmul(
                    p2r, w2x[:, 0, hc], o1i[:, hc], start=False, stop=last
                )
                nc.tensor.matmul(
                    p2i, w2x[:, 1, hc], o1i[:, hc], start=(hc == 0), stop=last
                )

            # --- bias + drain + store (transposed; host fixes layout) ---
            otr = outp.tile([128, ROWS], f32, tag="ot")
            nc.vector.tensor_scalar_add(otr, p2r, b2t[:, 0, j : j + 1])
            nc.sync.dma_start(out=out[j, 0], in_=otr)
            oti = outp.tile([128, ROWS], f32, tag="ot")
            nc.vector.tensor_scalar_add(oti, p2i, b2t[:, 1, j : j + 1])
            nc.sync.dma_start(out=out[j, 1], in_=oti)

    if not nc.is_finalized():
        nc.finalize()
    return nc


def _shard_inputs(x_real, x_imag, w1, b1, w2, b2):
    in_maps = []
    for jg in range(NJG):
        for rg in range(NRG):
            js = slice(jg * JL, (jg + 1) * JL)
            bs = slice(rg * BL, (rg + 1) * BL)
            # [BL, I, JL, K] -> [JL, K, BL*I]: kernel wants x pre-transposed
            xr_s = np.ascontiguousarray(
                x_real[bs, :, js, :].transpose(2, 3, 0, 1).reshape(JL, K, ROWS)
            )
            xi_s = np.ascontiguousarray(
                x_imag[bs, :, js, :].transpose(2, 3, 0, 1).reshape(JL, K, ROWS)
            )
            in_maps.append(
                {
                    "xr": xr_s,
                    "xi": xi_s,
                    "xs": xr_s + xi_s,
                    "w1": np.ascontiguousarray(w1[:, js]),
                    "b1": np.ascontiguousarray(b1[:, js]),
                    "w2": np.ascontiguousarray(w2[:, js]),
                    "b2": np.ascontiguousarray(b2[:, js]),
                }
            )
    return in_maps


def _gather(results):
    out = np.empty((B, I, J, K), np.complex64)
    idx = 0
    for jg in range(NJG):
        for rg in range(NRG):
            js = slice(jg * JL, (jg + 1) * JL)
            bs = slice(rg * BL, (rg + 1) * BL)
            o = np.asarray(results[idx]["out"], dtype=np.float32)  # [13,2,128,512]
            oc = (o[:, 0] + 1j * o[:, 1]).astype(np.complex64)  # [13,128,512]
            # [j, k, rows] -> [rows, j, k] -> [BL, I, JL, K]
            out[bs, :, js, :] = oc.transpose(2, 0, 1).reshape(BL, I, JL, K)
            idx += 1
    return out


def run(trace=False, **inputs):
    from concourse.bass_utils import run_bass_kernel_spmd

    if "nc" not in _cache:
        _cache["nc"] = _build_nc()
    in_maps = _shard_inputs(
        np.asarray(inputs["x_real"], np.float32),
        np.asarray(inputs["x_imag"], np.float32),
        np.asarray(inputs["w1"], np.float32),
        np.asarray(inputs["b1"], np.float32),
        np.asarray(inputs["w2"], np.float32),
        np.asarray(inputs["b2"], np.float32),
    )
    res = run_bass_kernel_spmd(_cache["nc"], in_maps, list(range(8)), trace=trace)
    return _gather(res.results), res


def kernel(**inputs):
    out, _ = run(trace=False, **inputs)
    return out



# revision 2
# speedup vs baseline: 3.0853x; 3.0853x over previous
"""Trainium2 Bass kernel for nn_MlpMixer_18966575579742 (bf16 rewrite).

Complex-valued per-frequency (j) MLP:
  o1r = gelu(xr@w1[0] - xi@w1[1] + b1[0]);  o1i = gelu(xi@w1[0] + xr@w1[1] + b1[1])
  o2r = o1r@w2[0] - o1i@w2[1] + b2[0];      o2i = o1i@w2[0] + o1i@w2[1] + b2[1]
  (note: o2i intentionally uses o1i with BOTH w2[0] and w2[1], as in the source)

Sharding over 8 cores: 2 j-halves (13 each) x 4 batch-quarters (B=32 -> 512 rows).

Key differences vs the fp32 baseline (289us):
  - ALL matmul operands are bf16 (fp32 matmul = 2 HW passes + 2x DMA bytes;
    tolerance is 2e-2 absmax so bf16's ~5e-3 is plenty). 312 passes/core.
  - L1 is the DIRECT 4-matmul complex product accumulated in PSUM
    (pre_r = w1[0]^T@xr + (-w1[1])^T@xi etc.), so the Gauss-trick's 156 DVE
    combine ops are gone entirely; GELU+bias reads PSUM directly on ScalarE.
  - L2 uses the algebraic identity o2r + o2i_pre = (o1r+o1i)@w2[0] [since
    o2i_pre = o1i@(w2[0]+w2[1])]: only 2 matmuls per h-chunk:
      T = o1i@(w2[0]+w2[1])   (= o2i pre-bias)
      S = (o1r+o1i)@w2[0]     (o2r = S - T + b2r)
    costing one bf16 DVE add per h-chunk (o1s = o1r + o1i).
  - per-j DMA is 3 big contiguous transfers (x-pair 2KB/part, weight-pack
    5KB/part incl. host-negated -w1[1] and host-summed w2[0]+w2[1], out-pair
    2KB/part); weight DMAs issue on gpsimd, x/out on sync.
  - L2 matmuls are emitted with a 2-slot lag behind L1 (pending deque) so the
    PE never waits on the ScalarE GELU -> DVE add chain.
  - PSUM: 4 banks rotate L1 pre-tiles (2 h-chunks in flight), 4 banks rotate
    L2 T/S accumulators (2 j in flight). Exactly 8.
  - a dummy GELU at kernel start pulls the ~2.7us ACT table load under the
    initial DMA wait.
"""

import sys

if "/opt/trn_rl_repo" not in sys.path:
    sys.path.insert(0, "/opt/trn_rl_repo")

from collections import deque

import numpy as np
from ml_dtypes import bfloat16

B, I, J, K, F = 128, 16, 26, 128, 4
H = K * F  # 512
NJG = 2  # j groups
NRG = 4  # row (batch) groups
JL = J // NJG  # 13 j per core
BL = B // NRG  # 32 batches per core
ROWS = BL * I  # 512 rows per core
NHC = H // 128  # 4 h-chunks
WCOLS = 5 * H  # w1[0] | w1[1] | -w1[1] | w2[0] | w2[0]+w2[1]

_cache = {}


def _build_nc():
    from contextlib import ExitStack

    import concourse.mybir as mybir
    import concourse.tile as tile
    from concourse import bacc
    from concourse.masks import make_identity

    f32 = mybir.dt.float32
    bf16 = mybir.dt.bfloat16
    nc = bacc.Bacc(None)

    # x pre-transposed on host: [j, k, rows*2] = [xr | xi]
    xp = nc.declare_dram_parameter("xp", [JL, K, 2 * ROWS], bf16, isOutput=False)
    # weight pack: [j, 128, 5*H]; first 3 slots partition=k, last 2 partition=h%128
    wp = nc.declare_dram_parameter("wp", [JL, 128, WCOLS], bf16, isOutput=False)
    b1 = nc.declare_dram_parameter("b1", [2, JL, H], f32, isOutput=False)
    b2 = nc.declare_dram_parameter("b2", [2, JL, K], f32, isOutput=False)
    # transposed output: [j, k', rows*2] = [real | imag]; host fixes layout
    out = nc.declare_dram_parameter("out", [JL, K, 2 * ROWS], bf16, isOutput=True)

    GELU = mybir.ActivationFunctionType.Gelu

    with tile.TileContext(nc) as tc, ExitStack() as ctx:
        const = ctx.enter_context(tc.tile_pool(name="const", bufs=1))
        xpool = ctx.enter_context(tc.tile_pool(name="xpool", bufs=3))
        wpool = ctx.enter_context(tc.tile_pool(name="wpool", bufs=3))
        o1p = ctx.enter_context(tc.tile_pool(name="o1p", bufs=2))
        srpp = ctx.enter_context(tc.tile_pool(name="srpp", bufs=2))
        outp = ctx.enter_context(tc.tile_pool(name="outp", bufs=3))
        ps1 = ctx.enter_context(tc.tile_pool(name="ps1", bufs=4, space="PSUM"))
        ps2 = ctx.enter_context(tc.tile_pool(name="ps2", bufs=4, space="PSUM"))

        identity = const.tile([128, 128], f32)
        make_identity(nc, identity)

        # warm the gelu table set while the first x/w DMAs are in flight
        warm = const.tile([128, 1], bf16)
        nc.scalar.activation(warm, identity[:, 0:1], GELU)

        # biases: clean row-major staging DMA, then PE-transpose on chip.
        b1s = const.tile([2 * JL * NHC, 128], f32)
        nc.gpsimd.dma_start(
            out=b1s, in_=b1.rearrange("c j (hc p) -> (c j hc) p", p=128)
        )
        b2s = const.tile([2 * JL, K], f32)
        nc.gpsimd.dma_start(out=b2s, in_=b2.rearrange("c j k -> (c j) k"))
        b1t = const.tile([128, 2, JL, NHC], f32)
        b2t = const.tile([128, 2, JL], f32)

        n1 = 2 * JL * NHC
        b1ps = ps2.tile([128, n1], f32, tag="ps2")
        nc.tensor.transpose(b1ps, b1s, identity[:n1, :n1])
        nc.vector.tensor_copy(b1t.rearrange("p c j hc -> p (c j hc)"), b1ps)
        n2 = 2 * JL
        b2ps = ps2.tile([128, n2], f32, tag="ps2")
        nc.tensor.transpose(b2ps, b2s, identity[:n2, :n2])
        nc.vector.tensor_copy(b2t.rearrange("p c j -> p (c j)"), b2ps)

        jstate = {}

        def start_j(j):
            xt = xpool.tile([128, 2 * ROWS], bf16, tag="xt")
            nc.sync.dma_start(out=xt, in_=xp[j])
            wt = wpool.tile([128, WCOLS], bf16, tag="wt")
            nc.gpsimd.dma_start(out=wt, in_=wp[j])
            jstate[j] = (xt, wt)

        TS = {}  # j -> (T, S) psum accumulators, allocated at first L2 pop

        def emit_L2(j, hc, wt, o1i, o1s):
            if hc == 0:
                T = ps2.tile([128, ROWS], f32, tag="ps2")
                S = ps2.tile([128, ROWS], f32, tag="ps2")
                TS[j] = (T, S)
            T, S = TS[j]
            c0 = 4 * H + hc * 128  # w2sum slot
            nc.tensor.matmul(
                T, wt[:, c0 : c0 + 128], o1i[:, hc],
                start=(hc == 0), stop=(hc == NHC - 1),
            )
            c1 = 3 * H + hc * 128  # w2[0] slot
            nc.tensor.matmul(
                S, wt[:, c1 : c1 + 128], o1s[:, hc],
                start=(hc == 0), stop=(hc == NHC - 1),
            )

        def emit_drain(j):
            T, S = TS.pop(j)
            ot = outp.tile([128, 2 * ROWS], bf16, tag="ot")
            # imag: T + b2i
            nc.vector.tensor_scalar_add(ot[:, ROWS:], T, b2t[:, 1, j : j + 1])
            # real: (S + b2r) - T
            srp = srpp.tile([128, ROWS], f32, tag="srp")
            nc.vector.tensor_scalar_add(srp, S, b2t[:, 0, j : j + 1])
            nc.vector.tensor_sub(ot[:, 0:ROWS], srp, T)
            nc.sync.dma_start(out=out[j], in_=ot)

        start_j(0)
        start_j(1)
        pend = deque()
        for j in range(JL):
            if j + 2 < JL:
                start_j(j + 2)
            xt, wt = jstate.pop(j)
            xr_ = xt[:, 0:ROWS]
            xi_ = xt[:, ROWS:]
            o1r = o1p.tile([128, NHC, ROWS], bf16, tag="o1r")
            o1i = o1p.tile([128, NHC, ROWS], bf16, tag="o1i")
            o1s = o1p.tile([128, NHC, ROWS], bf16, tag="o1s")
            for hc in range(NHC):
                hb = hc * 128
                pr = ps1.tile([128, ROWS], f32, tag="ps1")
                pi = ps1.tile([128, ROWS], f32, tag="ps1")
                # pre_r = w1[0]^T @ xr + (-w1[1])^T @ xi
                # pre_i = w1[0]^T @ xi +   w1[1]^T @ xr
                nc.tensor.matmul(
                    pr, wt[:, hb : hb + 128], xr_, start=True, stop=False
                )
                nc.tensor.matmul(
                    pi, wt[:, hb : hb + 128], xi_, start=True, stop=False
                )
                nc.tensor.matmul(
                    pi, wt[:, H + hb : H + hb + 128], xr_, start=False, stop=True
                )
                nc.tensor.matmul(
                    pr, wt[:, 2 * H + hb : 2 * H + hb + 128], xi_,
                    start=False, stop=True,
                )
                nc.scalar.activation(
                    o1r[:, hc], pr, GELU, bias=b1t[:, 0, j, hc : hc + 1]
                )
                nc.scalar.activation(
                    o1i[:, hc], pi, GELU, bias=b1t[:, 1, j, hc : hc + 1]
                )
                nc.vector.tensor_add(o1s[:, hc], o1r[:, hc], o1i[:, hc])
                pend.append((j, hc, wt, o1i, o1s))
                while len(pend) > 2:
                    pj, phc, pwt, po1i, po1s = pend.popleft()
                    emit_L2(pj, phc, pwt, po1i, po1s)
                    if phc == NHC - 1:
                        emit_drain(pj)
        while pend:
            pj, phc, pwt, po1i, po1s = pend.popleft()
            emit_L2(pj, phc, pwt, po1i, po1s)
            if phc == NHC - 1:
                emit_drain(pj)

    if not nc.is_finalized():
        nc.finalize()
    return nc


def _prep_shards(x_real, x_imag, w1, b1, w2, b2):
    """Host-side packing. Returns one input map per core (8 = 2 jg x 4 rg)."""
    wpks, b1l, b2l = [], [], []
    for jg in range(NJG):
        js = slice(jg * JL, (jg + 1) * JL)
        w10 = w1[0, js]  # [JL, K, H] partition=k
        w11 = w1[1, js]
        w2z = w2[0, js]  # [JL, H, K]
        w2sum = w2[0, js] + w2[1, js]
        # [JL, H, K] -> [JL, 128, NHC*K] with partition = h % 128
        w2z_r = (
            w2z.reshape(JL, NHC, 128, K).transpose(0, 2, 1, 3).reshape(JL, 128, NHC * K)
        )
        w2s_r = (
            w2sum.reshape(JL, NHC, 128, K)
            .transpose(0, 2, 1, 3)
            .reshape(JL, 128, NHC * K)
        )
        wpk = np.concatenate([w10, w11, -w11, w2z_r, w2s_r], axis=2).astype(bfloat16)
        wpks.append(np.ascontiguousarray(wpk))
        b1l.append(np.ascontiguousarray(b1[:, js]))
        b2l.append(np.ascontiguousarray(b2[:, js]))

    in_maps = []
    for jg in range(NJG):
        js = slice(jg * JL, (jg + 1) * JL)
        for rg in range(NRG):
            bs = slice(rg * BL, (rg + 1) * BL)
            # [BL, I, JL, K] -> [JL, K, BL*I]
            xr_s = x_real[bs, :, js, :].transpose(2, 3, 0, 1).reshape(JL, K, ROWS)
            xi_s = x_imag[bs, :, js, :].transpose(2, 3, 0, 1).reshape(JL, K, ROWS)
            xpk = np.concatenate([xr_s, xi_s], axis=2).astype(bfloat16)
            in_maps.append(
                {
                    "xp": np.ascontiguousarray(xpk),
                    "wp": wpks[jg],
                    "b1": b1l[jg],
                    "b2": b2l[jg],
                }
            )
    return in_maps


def _gather(results):
    out = np.empty((B, I, J, K), np.complex64)
    idx = 0
    for jg in range(NJG):
        for rg in range(NRG):
            js = slice(jg * JL, (jg + 1) * JL)
            bs = slice(rg * BL, (rg + 1) * BL)
            o = np.asarray(results[idx]["out"]).astype(np.float32)  # [13,128,1024]
            oc = (o[:, :, :ROWS] + 1j * o[:, :, ROWS:]).astype(np.complex64)
            # [j, k, rows] -> [rows, j, k] -> [BL, I, JL, K]
            out[bs, :, js, :] = oc.transpose(2, 0, 1).reshape(BL, I, JL, K)
            idx += 1
    return out


def run(trace=False, **inputs):
    from concourse.bass_utils import run_bass_kernel_spmd

    if "nc" not in _cache:
        _cache["nc"] = _build_nc()
    in_maps = _prep_shards(
        np.asarray(inputs["x_real"], np.float32),
        np.asarray(inputs["x_imag"], np.float32),
        np.asarray(inputs["w1"], np.float32),
        np.asarray(inputs["b1"], np.float32),
        np.asarray(inputs["w2"], np.float32),
        np.asarray(inputs["b2"], np.float32),
    )
    res = run_bass_kernel_spmd(_cache["nc"], in_maps, list(range(8)), trace=trace)
    return _gather(res.results), res


def kernel(**inputs):
    out, _ = run(trace=False, **inputs)
    return out
